# revision 14
# baseline (speedup 1.0000x reference)
"""Multi-head causal attention (B=2, S=2048, D=2048, 16 heads) on 8 TRN2 cores.

Sharding: 2-way batch parallel x 4-way head tensor-parallel (4 heads/core).
Each core computes q/k/v projections for its 4 heads, causal softmax
attention, and a partial o-projection; the host sums the 4 partials per batch.

All tensors are fp16 on chip (fp32 accumulation in PSUM): fp16 matmuls run at
the same PE rate as f32r but halve DMA traffic, SBUF footprint, and the
per-matmul stationary-weight-load cost, while keeping ~2^-11 precision
(everything here is O(1) so fp16 range is ample).

Host pre-transposes x and the weight slices so every on-chip matmul has its
contraction dim on SBUF partitions (no on-chip transposes at all):
  xT  [D, S]   = x[b].T
  wqT [D, JC]  = wq[j0:j0+512, :].T     (same wkT, wvT)
  woT [JC, D]  = wo[:, j0:j0+512].T

On-chip dataflow (per core):
  phase 1: x_sb, wq/wk/wv all fp16-resident (no respills, no restreaming);
           q/k/v projections with kT/qT/v_all kept resident in SBUF.
           v is dt-outer (4 parallel PSUM banks) so PE starts as soon as the
           first x / wv strips land.
  phase 2 (per head, per 512-wide i-chunk):
           a) scoresT[j,i] = k_h @ q_h.T per j-tile pair into a wide PSUM
              region, ONE exp per pair on ScalarE (scale fused), diagonal
              j-tiles trimmed to i >= j and masked with a triangular mask;
           b) denominator accumulated on DVE (fp16 2x mode) instead of PE
              ones-matmuls; a single ones-matmul broadcasts the summed row
              across partitions at the end of each chunk;
           c) attT[dv,i] += v_h[j,dv]-stationary @ probsT[j,i] over j-tiles;
              normalize with DVE reciprocal + multiply.
  phase 3: out[s,m] partial = sum_h attT_h.T @ woT_h, fp16 out to DRAM
           (host upcasts and sums the 4 partials per batch).
"""

import math

import numpy as np

B, S, D = 2, 2048, 2048
HEADS, HEAD_DIM = 16, 128
P = 128
JC = 512          # per-core projection width (4 heads x 128)
SC = 512          # s-chunk / matmul moving width
DT = D // P       # 16 contraction tiles
NSC = S // SC     # 4 s-chunks
NST = S // P      # 16 s-tiles
HPC = 4           # heads per core
N_CORES = 8
SCALE = 1.0 / math.sqrt(HEAD_DIM)

_NC_CACHE = {}


def build_module(reps=1, phases=(1, 2, 3), den="dve", p1_parts="vkq"):
    """Build + compile the (single-program SPMD) Bass module once.

    reps>1 repeats the whole kernel body inside one NEFF (for timing:
    differencing per-call wall times cancels the fixed dispatch overhead).
    phases: which kernel phases to include (timing experiments only).
    den: "dve" accumulates the softmax denominator on DVE; "pe" uses
    per-j-tile ones-matmuls on the tensor engine (baseline behaviour).
    p1_parts: subset of "vkq" — which projections phase 1 runs (timing
    experiments only; correctness requires all three).

    All PE accumulation runs as 4 round-robin chains across PSUM banks:
    back-to-back accumulating matmuls into the SAME bank measure ~1.4-2.0
    cyc/row on HW, while 4-way bank rotation sustains ~1.0 cyc/row.
    """
    phases = tuple(phases)
    key = (reps, phases, den, p1_parts)
    if key in _NC_CACHE:
        return _NC_CACHE[key]

    from contextlib import ExitStack

    import concourse.tile as tile
    from concourse import bacc
    import concourse.mybir as mybir

    f16 = mybir.dt.float16
    f32 = mybir.dt.float32
    FT = mybir.ActivationFunctionType

    nc = bacc.Bacc(
        "TRN2", target_bir_lowering=False, debug=False, num_devices=N_CORES
    )

    xT = nc.dram_tensor("xT", [D, S], f16, kind="ExternalInput").ap()
    wqT = nc.dram_tensor("wqT", [D, JC], f16, kind="ExternalInput").ap()
    wkT = nc.dram_tensor("wkT", [D, JC], f16, kind="ExternalInput").ap()
    wvT = nc.dram_tensor("wvT", [D, JC], f16, kind="ExternalInput").ap()
    woT = nc.dram_tensor("woT", [JC, D], f16, kind="ExternalInput").ap()
    # mask[j, c] = 1 iff j <= c : causal triangle for a diagonal 128-block
    mask = nc.dram_tensor("mask", [P, P], f16, kind="ExternalInput").ap()
    ones = nc.dram_tensor("ones", [P, P], f16, kind="ExternalInput").ap()
    out = nc.dram_tensor("out", [S, D], f16, kind="ExternalOutput").ap()

    with tile.TileContext(nc) as tc, ExitStack() as ctx:
        consts = ctx.enter_context(tc.tile_pool(name="consts", bufs=1))

        mask_sb = consts.tile([P, P], f16, tag="mask", name="mask_sb")
        nc.sync.dma_start(mask_sb, mask)
        ones_sb = consts.tile([P, P], f16, tag="ones", name="ones_sb")
        nc.sync.dma_start(ones_sb, ones)

        xT_r = xT.rearrange("(dt p) s -> p dt s", p=P)
        wqT_r = wqT.rearrange("(dt p) j -> p dt j", p=P)
        wkT_r = wkT.rearrange("(dt p) j -> p dt j", p=P)
        wvT_r = wvT.rearrange("(dt p) j -> p dt j", p=P)
        woT_r = woT.rearrange("(hh p) m -> p hh m", p=P)

        # All SBUF pools are persistent across reps: a new rep's tile (same
        # tag) only waits for the PREVIOUS rep's readers of that slot, so
        # rep N+1's x/weight DMAs prefetch during rep N's phases 2-3.  The
        # SP DMA queue carries ONLY x (so its triggers aren't queued behind
        # the previous rep's output writes); weights / wo / out go on the
        # ACT queue.
        kvpool = ctx.enter_context(tc.tile_pool(name="kvpool", bufs=1))
        xwpool = ctx.enter_context(tc.tile_pool(name="xwpool", bufs=1))
        opool = ctx.enter_context(tc.tile_pool(name="opool", bufs=1))
        attp = ctx.enter_context(tc.tile_pool(name="attp", bufs=1))
        # den="pe" keeps every pair's probs alive until the chunk-end ones-
        # matmul loop, so it needs deeper prob buffering
        ppool = ctx.enter_context(
            tc.tile_pool(name="ppool", bufs=(2 if den == "dve" else 9))
        )
        dpool = ctx.enter_context(tc.tile_pool(name="dpool", bufs=1))
        rpool = ctx.enter_context(tc.tile_pool(name="rpool", bufs=1))
        ostage = ctx.enter_context(tc.tile_pool(name="ostage", bufs=2))

        for _rep in range(reps):
            with ExitStack() as prep:
                # kT/qT/v stay resident across phases 1-2 (48KB/partition)
                kT_all = kvpool.tile([P, HPC, S], f16, tag="kT", name="kT_all")
                qT_all = kvpool.tile([P, HPC, S], f16, tag="qT", name="qT_all")
                v_all = kvpool.tile([P, NST, JC], f16, tag="v", name="v_all")

                # ---------- Phase 1: q/k/v projections ----------
                with ExitStack() as p1:
                    psum1 = p1.enter_context(
                        tc.tile_pool(name="psum1", bufs=1, space="PSUM")
                    )

                    # everything fp16-resident: x 64KB, weights 16KB each
                    x_sb = xwpool.tile([P, DT, S], f16, tag="x", name="x_sb")
                    wq_sb = xwpool.tile([P, DT, JC], f16, tag="wq", name="wq_sb")
                    wk_sb = xwpool.tile([P, DT, JC], f16, tag="wk", name="wk_sb")
                    wv_sb = xwpool.tile([P, DT, JC], f16, tag="wv", name="wv_sb")

                    for sc in range(NSC):
                        scs = slice(sc * SC, (sc + 1) * SC)
                        for q4 in range(4):
                            dts = slice(q4 * 4, (q4 + 1) * 4)
                            nc.sync.dma_start(
                                x_sb[:, dts, scs], xT_r[:, dts, scs]
                            )
                        if sc == 0:
                            # wv in per-dt strips so the dt-outer v matmuls
                            # start as soon as the first strips land
                            for dt in range(DT):
                                nc.scalar.dma_start(
                                    wv_sb[:, dt, :], wvT_r[:, dt, :]
                                )

                        # v projection (dt-outer, 4 parallel PSUM banks)
                        if "v" in p1_parts:
                            ps_v = [
                                psum1.tile(
                                    [P, JC], f32, tag=f"pv{t}", bufs=1,
                                    name="ps_v"
                                )
                                for t in range(4)
                            ]
                            for dt in range(DT):
                                for t in range(4):
                                    nc.tensor.matmul(
                                        ps_v[t],
                                        lhsT=x_sb[
                                            :, dt, sc * SC + t * P:
                                            sc * SC + (t + 1) * P
                                        ],
                                        rhs=wv_sb[:, dt, :],
                                        start=(dt == 0),
                                        stop=(dt == DT - 1),
                                    )
                            for t in range(4):
                                nc.vector.tensor_copy(
                                    v_all[:, sc * 4 + t, :], ps_v[t]
                                )
                        if sc == 0:
                            # k/q weights next on the ACT DMA queue (k first:
                            # it's the next consumer)
                            for q4 in range(4):
                                dts = slice(q4 * 4, (q4 + 1) * 4)
                                nc.scalar.dma_start(
                                    wk_sb[:, dts, :], wkT_r[:, dts, :]
                                )
                            for q4 in range(4):
                                dts = slice(q4 * 4, (q4 + 1) * 4)
                                nc.scalar.dma_start(
                                    wq_sb[:, dts, :], wqT_r[:, dts, :]
                                )

                        # k projection: dt-outer over 4 round-robin chains
                        # (one per head) so consecutive matmuls rotate banks
                        if "k" in p1_parts:
                            ps_k = [
                                psum1.tile(
                                    [P, SC], f32, tag=f"kq{t}", bufs=1,
                                    name="ps_k"
                                )
                                for t in range(4)
                            ]
                            for dt in range(DT):
                                for t in range(4):
                                    nc.tensor.matmul(
                                        ps_k[t],
                                        lhsT=wk_sb[:, dt, t * P:(t + 1) * P],
                                        rhs=x_sb[:, dt, scs],
                                        start=(dt == 0),
                                        stop=(dt == DT - 1),
                                    )
                            for t in range(4):
                                nc.vector.tensor_copy(kT_all[:, t, scs], ps_k[t])

                        # q projection, same shape; reuses the v chains' bank
                        # tags (free by now), copies on ACT to keep DVE free
                        if "q" in p1_parts:
                            ps_q = [
                                psum1.tile(
                                    [P, SC], f32, tag=f"pv{t}", bufs=1,
                                    name="ps_q"
                                )
                                for t in range(4)
                            ]
                            for dt in range(DT):
                                for t in range(4):
                                    nc.tensor.matmul(
                                        ps_q[t],
                                        lhsT=wq_sb[:, dt, t * P:(t + 1) * P],
                                        rhs=x_sb[:, dt, scs],
                                        start=(dt == 0),
                                        stop=(dt == DT - 1),
                                    )
                            for t in range(4):
                                nc.scalar.copy(qT_all[:, t, scs], ps_q[t])

                # ---------- Phase 2/3 pools ----------
                if 2 not in phases:
                    continue
                with ExitStack() as p2:
                    p2a = p2.enter_context(ExitStack())
                    psum2 = p2a.enter_context(
                        tc.tile_pool(name="psum2", bufs=2, space="PSUM")
                    )

                    # Phase 3 weights prefetched on the ACT DMA queue, one
                    # 1MB piece per head iteration
                    woTs = opool.tile([P, HPC, D], f16, tag="wo", name="woTs")

                    # ---------- Phase 2: causal attention per head ----------
                    attTs = []
                    for h in range(HPC):
                        nc.scalar.dma_start(woTs[:, h, :], woT_r[:, h, :])
                        attT = attp.tile(
                            [P, S], f16, tag=f"attT{h}", name=f"attT_{h}"
                        )
                        attTs.append(attT)

                        for ic in range(NSC):
                            njt = 4 * ic + 4  # causal: j-tiles 0..njt-1
                            offs = [
                                max(0, (jt - 4 * ic) * P) for jt in range(njt)
                            ]
                            qh = qT_all[:, h, ic * SC:(ic + 1) * SC]

                            ps_pv = psum2.tile(
                                [P, SC], f32, tag="pv", name="ps_pv"
                            )
                            if den == "pe":
                                ps_den = psum2.tile(
                                    [P, SC], f32, tag="den", name="ps_den"
                                )
                            else:
                                den_acc = dpool.tile(
                                    [P, SC], f16, tag="da", name="den_acc"
                                )

                            # j-tile pairs: scores into a 2-bank psum region,
                            # ONE exp per pair (halves ScalarE instr overhead)
                            pts = []  # (wide tile, half index u) per jt
                            for g in range(njt // 2):
                                ps_s = psum2.tile(
                                    [P, 2 * SC], f32, tag="score", bufs=2,
                                    name="ps_s",
                                )
                                ptw = ppool.tile(
                                    [P, 2 * SC], f16, tag="prob", name="pt"
                                )
                                off0 = offs[2 * g]
                                for u in range(2):
                                    jt = 2 * g + u
                                    pts.append((ptw, u))
                                    # write from the pair's min offset so the
                                    # single wide exp never reads unwritten
                                    # psum; den/pv still slice from offs[jt]
                                    nc.tensor.matmul(
                                        ps_s[:, u * SC + off0:(u + 1) * SC],
                                        lhsT=kT_all[
                                            :, h, jt * P:(jt + 1) * P
                                        ],
                                        rhs=qh[:, off0:],
                                        start=True,
                                        stop=True,
                                    )
                                if off0 == 0:
                                    nc.scalar.activation(
                                        ptw, ps_s, FT.Exp, scale=SCALE
                                    )
                                else:
                                    # columns [SC, SC+off0) are unwritten;
                                    # exp each half separately
                                    nc.scalar.activation(
                                        ptw[:, off0:SC], ps_s[:, off0:SC],
                                        FT.Exp, scale=SCALE,
                                    )
                                    nc.scalar.activation(
                                        ptw[:, SC + off0:],
                                        ps_s[:, SC + off0:],
                                        FT.Exp, scale=SCALE,
                                    )
                                for u in range(2):
                                    jt = 2 * g + u
                                    off = offs[jt]
                                    if jt >= 4 * ic:
                                        # triangular mask on diagonal block
                                        nc.vector.tensor_mul(
                                            out=ptw[
                                                :,
                                                u * SC + off:u * SC + off + P,
                                            ],
                                            in0=ptw[
                                                :,
                                                u * SC + off:u * SC + off + P,
                                            ],
                                            in1=mask_sb,
                                        )
                                    if den == "dve":
                                        # denominator partial sums on DVE
                                        # (fp16 2x mode)
                                        if jt == 0:
                                            nc.vector.tensor_copy(
                                                den_acc, ptw[:, :SC]
                                            )
                                        else:
                                            nc.vector.tensor_add(
                                                out=den_acc[:, off:],
                                                in0=den_acc[:, off:],
                                                in1=ptw[
                                                    :, u * SC + off:
                                                    (u + 1) * SC
                                                ],
                                            )
                                    # c) attT[dv,i] += v_h[j,dv] @ probsT[j,i]
                                    nc.tensor.matmul(
                                        ps_pv[:, off:],
                                        lhsT=v_all[
                                            :, jt,
                                            h * HEAD_DIM:(h + 1) * HEAD_DIM
                                        ],
                                        rhs=ptw[:, u * SC + off:(u + 1) * SC],
                                        start=(jt == 0),
                                        stop=(jt == njt - 1),
                                        skip_group_check=True,
                                    )

                            # b) denominator -> broadcast across partitions
                            if den == "pe":
                                for jt in range(njt):
                                    off = offs[jt]
                                    ptw, u = pts[jt]
                                    nc.tensor.matmul(
                                        ps_den[:, off:],
                                        lhsT=ones_sb,
                                        rhs=ptw[:, u * SC + off:(u + 1) * SC],
                                        start=(jt == 0),
                                        stop=(jt == njt - 1),
                                        skip_group_check=True,
                                    )
                            else:
                                ps_den = psum2.tile(
                                    [P, SC], f32, tag="den", name="ps_den"
                                )
                                nc.tensor.matmul(
                                    ps_den,
                                    lhsT=ones_sb,
                                    rhs=den_acc,
                                    start=True,
                                    stop=True,
                                )
                            rec = rpool.tile(
                                [P, SC], f32, tag="rec", name="rec"
                            )
                            nc.vector.reciprocal(rec, ps_den)
                            nc.vector.tensor_mul(
                                out=attT[:, ic * SC:(ic + 1) * SC],
                                in0=ps_pv,
                                in1=rec,
                            )

                    # ---------- Phase 3: partial o-projection ----------
                    # own PSUM scope: 4 round-robin chains (one per 512-wide
                    # output chunk), hh-inner so consecutive matmuls rotate
                    # across 4 banks; double-buffered for st overlap
                    p2a.close()
                    psum3 = p2.enter_context(
                        tc.tile_pool(name="psum3", bufs=2, space="PSUM")
                    )
                    # out writes go on the ACT queue (right after each og
                    # copy) so the SP queue stays dedicated to x prefetch
                    for st in range(NST if 3 in phases else 0):
                        og = ostage.tile([P, D], f16, tag="og", name="og")
                        ps_o = [
                            psum3.tile(
                                [P, SC], f32, tag=f"o{mc}", bufs=2, name="ps_o"
                            )
                            for mc in range(4)
                        ]
                        for hh in range(HPC):
                            for mc in range(4):
                                nc.tensor.matmul(
                                    ps_o[mc],
                                    lhsT=attTs[hh][:, st * P:(st + 1) * P],
                                    rhs=woTs[:, hh, mc * SC:(mc + 1) * SC],
                                    start=(hh == 0),
                                    stop=(hh == HPC - 1),
                                )
                        for mc in range(4):
                            # copies alternate DVE/ACT so neither serializes
                            if mc % 2 == 0:
                                nc.vector.tensor_copy(
                                    og[:, mc * SC:(mc + 1) * SC], ps_o[mc]
                                )
                            else:
                                nc.scalar.copy(
                                    og[:, mc * SC:(mc + 1) * SC], ps_o[mc]
                                )
                        nc.scalar.dma_start(out[st * P:(st + 1) * P, :], og)

    nc.compile()
    _NC_CACHE[key] = nc
    return nc


def make_in_maps(x, wq, wk, wv, wo):
    x = np.asarray(x, dtype=np.float16)
    wq = np.asarray(wq, dtype=np.float16)
    wk = np.asarray(wk, dtype=np.float16)
    wv = np.asarray(wv, dtype=np.float16)
    wo = np.asarray(wo, dtype=np.float16)
    # mask[j, c] = 1 iff key j visible to query c within a diagonal block
    causal = np.triu(np.ones((P, P), dtype=np.float16))
    ones = np.ones((P, P), dtype=np.float16)
    in_maps = []
    for c in range(N_CORES):
        b, g = divmod(c, HPC)
        j0 = g * JC
        in_maps.append(
            {
                "xT": np.ascontiguousarray(x[b].T),
                "wqT": np.ascontiguousarray(wq[j0:j0 + JC].T),
                "wkT": np.ascontiguousarray(wk[j0:j0 + JC].T),
                "wvT": np.ascontiguousarray(wv[j0:j0 + JC].T),
                "woT": np.ascontiguousarray(wo[:, j0:j0 + JC].T),
                "mask": causal,
                "ones": ones,
            }
        )
    return in_maps


def combine_outputs(results):
    out = np.zeros((B, S, D), dtype=np.float32)
    for c in range(N_CORES):
        out[c // HPC] += np.asarray(results[c]["out"], dtype=np.float32)
    return out


def kernel(x, wq, wk, wv, wo):
    from concourse.bass_utils import run_bass_kernel_spmd

    nc = build_module()
    in_maps = make_in_maps(x, wq, wk, wv, wo)
    res = run_bass_kernel_spmd(nc, in_maps, list(range(N_CORES)))
    return combine_outputs(res.results)


# revision 19
# speedup vs baseline: 1.0404x; 1.0404x over previous
"""Multi-head causal attention (B=2, S=2048, D=2048, 16 heads) on 8 TRN2 cores.

Sharding: 2-way batch parallel x 4-way head tensor-parallel (4 heads/core).
Each core computes q/k/v projections for its 4 heads, causal softmax
attention, and a partial o-projection; the host sums the 4 partials per batch.

All tensors are fp16 on chip (fp32 accumulation in PSUM): fp16 matmuls run at
the same PE rate as f32r but halve DMA traffic, SBUF footprint, and the
per-matmul stationary-weight-load cost, while keeping ~2^-11 precision
(everything here is O(1) so fp16 range is ample).

Host pre-transposes x and the weight slices so every on-chip matmul has its
contraction dim on SBUF partitions (no on-chip transposes at all):
  xT  [D, S]   = x[b].T
  wqT [D, JC]  = wq[j0:j0+512, :].T     (same wkT, wvT)
  woT [JC, D]  = wo[:, j0:j0+512].T

On-chip dataflow (per core):
  phase 1: x_sb, wq/wk/wv all fp16-resident (no respills, no restreaming);
           q/k/v projections with kT/qT/v_all kept resident in SBUF.
           v is dt-outer (4 parallel PSUM banks) so PE starts as soon as the
           first x / wv strips land.
  phase 2 (per head, per 512-wide i-chunk):
           a) scoresT[j,i] = k_h @ q_h.T per j-tile pair into a wide PSUM
              region, ONE exp per pair on ScalarE (scale fused), diagonal
              j-tiles trimmed to i >= j and masked with a triangular mask;
           b) denominator accumulated on DVE (fp16 2x mode) instead of PE
              ones-matmuls; a single ones-matmul broadcasts the summed row
              across partitions at the end of each chunk;
           c) attT[dv,i] += v_h[j,dv]-stationary @ probsT[j,i] over j-tiles;
              normalize with DVE reciprocal + multiply.
  phase 3: out[s,m] partial = sum_h attT_h.T @ woT_h, fp16 out to DRAM
           (host upcasts and sums the 4 partials per batch).
"""

import math

import numpy as np

B, S, D = 2, 2048, 2048
HEADS, HEAD_DIM = 16, 128
P = 128
JC = 512          # per-core projection width (4 heads x 128)
SC = 512          # s-chunk / matmul moving width
DT = D // P       # 16 contraction tiles
NSC = S // SC     # 4 s-chunks
NST = S // P      # 16 s-tiles
HPC = 4           # heads per core
N_CORES = 8
SCALE = 1.0 / math.sqrt(HEAD_DIM)

_NC_CACHE = {}


def build_module(reps=1, phases=(1, 2, 3), den="pe", p1_parts="vkq"):
    """Build + compile the (single-program SPMD) Bass module once.

    reps>1 repeats the whole kernel body inside one NEFF (for timing:
    differencing per-call wall times cancels the fixed dispatch overhead).
    phases: which kernel phases to include (timing experiments only).
    den: "dve" accumulates the softmax denominator on DVE; "pe" uses
    per-j-tile ones-matmuls on the tensor engine (baseline behaviour).
    p1_parts: subset of "vkq" — which projections phase 1 runs (timing
    experiments only; correctness requires all three).

    All PE accumulation runs as 4 round-robin chains across PSUM banks:
    back-to-back accumulating matmuls into the SAME bank measure ~1.4-2.0
    cyc/row on HW, while 4-way bank rotation sustains ~1.0 cyc/row.
    """
    phases = tuple(phases)
    key = (reps, phases, den, p1_parts)
    if key in _NC_CACHE:
        return _NC_CACHE[key]

    from contextlib import ExitStack

    import concourse.tile as tile
    from concourse import bacc
    import concourse.mybir as mybir

    f16 = mybir.dt.float16
    f32 = mybir.dt.float32
    FT = mybir.ActivationFunctionType

    nc = bacc.Bacc(
        "TRN2", target_bir_lowering=False, debug=False, num_devices=N_CORES
    )

    xT = nc.dram_tensor("xT", [D, S], f16, kind="ExternalInput").ap()
    wqT = nc.dram_tensor("wqT", [D, JC], f16, kind="ExternalInput").ap()
    wkT = nc.dram_tensor("wkT", [D, JC], f16, kind="ExternalInput").ap()
    wvT = nc.dram_tensor("wvT", [D, JC], f16, kind="ExternalInput").ap()
    woT = nc.dram_tensor("woT", [JC, D], f16, kind="ExternalInput").ap()
    # mask[j, c] = 1 iff j <= c : causal triangle for a diagonal 128-block
    mask = nc.dram_tensor("mask", [P, P], f16, kind="ExternalInput").ap()
    ones = nc.dram_tensor("ones", [P, P], f16, kind="ExternalInput").ap()
    out = nc.dram_tensor("out", [S, D], f16, kind="ExternalOutput").ap()

    with tile.TileContext(nc) as tc, ExitStack() as ctx:
        consts = ctx.enter_context(tc.tile_pool(name="consts", bufs=1))

        mask_sb = consts.tile([P, P], f16, tag="mask", name="mask_sb")
        nc.sync.dma_start(mask_sb, mask)
        ones_sb = consts.tile([P, P], f16, tag="ones", name="ones_sb")
        nc.sync.dma_start(ones_sb, ones)

        xT_r = xT.rearrange("(dt p) s -> p dt s", p=P)
        wqT_r = wqT.rearrange("(dt p) j -> p dt j", p=P)
        wkT_r = wkT.rearrange("(dt p) j -> p dt j", p=P)
        wvT_r = wvT.rearrange("(dt p) j -> p dt j", p=P)
        woT_r = woT.rearrange("(hh p) m -> p hh m", p=P)

        for _rep in range(reps):
            with ExitStack() as prep:
                # kT/qT/v stay resident across phases 1-2 (48KB/partition)
                kvpool = prep.enter_context(tc.tile_pool(name="kvpool", bufs=1))
                kT_all = kvpool.tile([P, HPC, S], f16, tag="kT", name="kT_all")
                qT_all = kvpool.tile([P, HPC, S], f16, tag="qT", name="qT_all")
                v_all = kvpool.tile([P, NST, JC], f16, tag="v", name="v_all")

                # ---------- Phase 1: q/k/v projections ----------
                with ExitStack() as p1:
                    xpool = p1.enter_context(tc.tile_pool(name="xpool", bufs=1))
                    wpool = p1.enter_context(tc.tile_pool(name="wpool", bufs=1))
                    psum1 = p1.enter_context(
                        tc.tile_pool(name="psum1", bufs=1, space="PSUM")
                    )

                    # everything fp16-resident: x 64KB, weights 16KB each
                    x_sb = xpool.tile([P, DT, S], f16, tag="x", name="x_sb")
                    wq_sb = wpool.tile([P, DT, JC], f16, tag="wq", name="wq_sb")
                    wk_sb = wpool.tile([P, DT, JC], f16, tag="wk", name="wk_sb")
                    wv_sb = wpool.tile([P, DT, JC], f16, tag="wv", name="wv_sb")

                    for sc in range(NSC):
                        scs = slice(sc * SC, (sc + 1) * SC)
                        for q4 in range(4):
                            dts = slice(q4 * 4, (q4 + 1) * 4)
                            nc.sync.dma_start(
                                x_sb[:, dts, scs], xT_r[:, dts, scs]
                            )
                        if sc == 0:
                            # wv in per-dt strips so the dt-outer v matmuls
                            # start as soon as the first strips land
                            for dt in range(DT):
                                nc.scalar.dma_start(
                                    wv_sb[:, dt, :], wvT_r[:, dt, :]
                                )

                        # v projection (dt-outer, 4 parallel PSUM banks)
                        if "v" in p1_parts:
                            ps_v = [
                                psum1.tile(
                                    [P, JC], f32, tag=f"pv{t}", bufs=1,
                                    name="ps_v"
                                )
                                for t in range(4)
                            ]
                            for dt in range(DT):
                                for t in range(4):
                                    nc.tensor.matmul(
                                        ps_v[t],
                                        lhsT=x_sb[
                                            :, dt, sc * SC + t * P:
                                            sc * SC + (t + 1) * P
                                        ],
                                        rhs=wv_sb[:, dt, :],
                                        start=(dt == 0),
                                        stop=(dt == DT - 1),
                                    )
                            for t in range(4):
                                nc.vector.tensor_copy(
                                    v_all[:, sc * 4 + t, :], ps_v[t]
                                )
                        if sc == 0:
                            # k/q weights next on the ACT DMA queue (k first:
                            # it's the next consumer)
                            for q4 in range(4):
                                dts = slice(q4 * 4, (q4 + 1) * 4)
                                nc.scalar.dma_start(
                                    wk_sb[:, dts, :], wkT_r[:, dts, :]
                                )
                            for q4 in range(4):
                                dts = slice(q4 * 4, (q4 + 1) * 4)
                                nc.scalar.dma_start(
                                    wq_sb[:, dts, :], wqT_r[:, dts, :]
                                )

                        # k projection: dt-outer over 4 round-robin chains
                        # (one per head) so consecutive matmuls rotate banks
                        if "k" in p1_parts:
                            ps_k = [
                                psum1.tile(
                                    [P, SC], f32, tag=f"kq{t}", bufs=1,
                                    name="ps_k"
                                )
                                for t in range(4)
                            ]
                            for dt in range(DT):
                                for t in range(4):
                                    nc.tensor.matmul(
                                        ps_k[t],
                                        lhsT=wk_sb[:, dt, t * P:(t + 1) * P],
                                        rhs=x_sb[:, dt, scs],
                                        start=(dt == 0),
                                        stop=(dt == DT - 1),
                                    )
                            for t in range(4):
                                nc.vector.tensor_copy(kT_all[:, t, scs], ps_k[t])

                        # q projection, same shape; reuses the v chains' bank
                        # tags (free by now), copies on ACT to keep DVE free
                        if "q" in p1_parts:
                            ps_q = [
                                psum1.tile(
                                    [P, SC], f32, tag=f"pv{t}", bufs=1,
                                    name="ps_q"
                                )
                                for t in range(4)
                            ]
                            for dt in range(DT):
                                for t in range(4):
                                    nc.tensor.matmul(
                                        ps_q[t],
                                        lhsT=wq_sb[:, dt, t * P:(t + 1) * P],
                                        rhs=x_sb[:, dt, scs],
                                        start=(dt == 0),
                                        stop=(dt == DT - 1),
                                    )
                            for t in range(4):
                                nc.scalar.copy(qT_all[:, t, scs], ps_q[t])

                # ---------- Phase 2/3 pools ----------
                if 2 not in phases:
                    continue
                with ExitStack() as p2:
                    opool = p2.enter_context(tc.tile_pool(name="opool", bufs=1))
                    attp = p2.enter_context(tc.tile_pool(name="attp", bufs=1))
                    ostage = p2.enter_context(
                        tc.tile_pool(name="ostage", bufs=2)
                    )
                    p2a = p2.enter_context(ExitStack())
                    ppool = p2a.enter_context(tc.tile_pool(name="ppool", bufs=4))
                    dpool = p2a.enter_context(tc.tile_pool(name="dpool", bufs=2))
                    rpool = p2a.enter_context(tc.tile_pool(name="rpool", bufs=2))
                    psum2 = p2a.enter_context(
                        tc.tile_pool(name="psum2", bufs=2, space="PSUM")
                    )

                    # Phase 3 weights prefetched on the idle SP DMA queue,
                    # one 1MB piece per head iteration
                    woTs = opool.tile([P, HPC, D], f16, tag="wo", name="woTs")

                    # ---------- Phase 2: causal attention per head ----------
                    attTs = []
                    for h in range(HPC):
                        nc.sync.dma_start(woTs[:, h, :], woT_r[:, h, :])
                        attT = attp.tile(
                            [P, S], f16, tag=f"attT{h}", name=f"attT_{h}"
                        )
                        attTs.append(attT)

                        for ic in range(NSC):
                            njt = 4 * ic + 4  # causal: j-tiles 0..njt-1
                            offs = [
                                max(0, (jt - 4 * ic) * P) for jt in range(njt)
                            ]
                            qh = qT_all[:, h, ic * SC:(ic + 1) * SC]

                            ps_pv = psum2.tile(
                                [P, SC], f32, tag="pv", name="ps_pv"
                            )
                            if den == "pe":
                                ps_den = psum2.tile(
                                    [P, SC], f32, tag="den", name="ps_den"
                                )
                            else:
                                den_acc = dpool.tile(
                                    [P, SC], f16, tag="da", name="den_acc"
                                )

                            # j-tile pairs: scores into a 2-bank psum region,
                            # ONE exp per pair (halves ScalarE instr overhead)
                            pts = []  # (wide tile, half index u) per jt
                            for g in range(njt // 2):
                                ps_s = psum2.tile(
                                    [P, 2 * SC], f32, tag="score", bufs=2,
                                    name="ps_s",
                                )
                                ptw = ppool.tile(
                                    [P, 2 * SC], f16, tag="prob", name="pt"
                                )
                                off0 = offs[2 * g]
                                for u in range(2):
                                    jt = 2 * g + u
                                    pts.append((ptw, u))
                                    # write from the pair's min offset so the
                                    # single wide exp never reads unwritten
                                    # psum; den/pv still slice from offs[jt]
                                    nc.tensor.matmul(
                                        ps_s[:, u * SC + off0:(u + 1) * SC],
                                        lhsT=kT_all[
                                            :, h, jt * P:(jt + 1) * P
                                        ],
                                        rhs=qh[:, off0:],
                                        start=True,
                                        stop=True,
                                    )
                                if off0 == 0:
                                    nc.scalar.activation(
                                        ptw, ps_s, FT.Exp, scale=SCALE
                                    )
                                else:
                                    # columns [SC, SC+off0) are unwritten;
                                    # exp each half separately
                                    nc.scalar.activation(
                                        ptw[:, off0:SC], ps_s[:, off0:SC],
                                        FT.Exp, scale=SCALE,
                                    )
                                    nc.scalar.activation(
                                        ptw[:, SC + off0:],
                                        ps_s[:, SC + off0:],
                                        FT.Exp, scale=SCALE,
                                    )
                                for u in range(2):
                                    jt = 2 * g + u
                                    off = offs[jt]
                                    if jt >= 4 * ic:
                                        # triangular mask on diagonal block
                                        nc.vector.tensor_mul(
                                            out=ptw[
                                                :,
                                                u * SC + off:u * SC + off + P,
                                            ],
                                            in0=ptw[
                                                :,
                                                u * SC + off:u * SC + off + P,
                                            ],
                                            in1=mask_sb,
                                        )
                                    if den == "dve":
                                        # denominator partial sums on DVE
                                        # (fp16 2x mode)
                                        if jt == 0:
                                            nc.vector.tensor_copy(
                                                den_acc, ptw[:, :SC]
                                            )
                                        else:
                                            nc.vector.tensor_add(
                                                out=den_acc[:, off:],
                                                in0=den_acc[:, off:],
                                                in1=ptw[
                                                    :, u * SC + off:
                                                    (u + 1) * SC
                                                ],
                                            )
                                    # c) attT[dv,i] += v_h[j,dv] @ probsT[j,i]
                                    nc.tensor.matmul(
                                        ps_pv[:, off:],
                                        lhsT=v_all[
                                            :, jt,
                                            h * HEAD_DIM:(h + 1) * HEAD_DIM
                                        ],
                                        rhs=ptw[:, u * SC + off:(u + 1) * SC],
                                        start=(jt == 0),
                                        stop=(jt == njt - 1),
                                        skip_group_check=True,
                                    )

                            # b) denominator -> broadcast across partitions
                            if den == "pe":
                                for jt in range(njt):
                                    off = offs[jt]
                                    ptw, u = pts[jt]
                                    nc.tensor.matmul(
                                        ps_den[:, off:],
                                        lhsT=ones_sb,
                                        rhs=ptw[:, u * SC + off:(u + 1) * SC],
                                        start=(jt == 0),
                                        stop=(jt == njt - 1),
                                        skip_group_check=True,
                                    )
                            else:
                                ps_den = psum2.tile(
                                    [P, SC], f32, tag="den", name="ps_den"
                                )
                                nc.tensor.matmul(
                                    ps_den,
                                    lhsT=ones_sb,
                                    rhs=den_acc,
                                    start=True,
                                    stop=True,
                                )
                            rec = rpool.tile(
                                [P, SC], f32, tag="rec", name="rec"
                            )
                            nc.vector.reciprocal(rec, ps_den)
                            nc.vector.tensor_mul(
                                out=attT[:, ic * SC:(ic + 1) * SC],
                                in0=ps_pv,
                                in1=rec,
                            )

                    # ---------- Phase 3: partial o-projection ----------
                    # own PSUM scope: 4 round-robin chains (one per 512-wide
                    # output chunk), hh-inner so consecutive matmuls rotate
                    # across 4 banks; double-buffered for st overlap
                    p2a.close()
                    psum3 = p2.enter_context(
                        tc.tile_pool(name="psum3", bufs=2, space="PSUM")
                    )
                    for st in range(NST if 3 in phases else 0):
                        og = ostage.tile([P, D], f16, tag="og", name="og")
                        ps_o = [
                            psum3.tile(
                                [P, SC], f32, tag=f"o{mc}", bufs=2, name="ps_o"
                            )
                            for mc in range(4)
                        ]
                        for hh in range(HPC):
                            for mc in range(4):
                                nc.tensor.matmul(
                                    ps_o[mc],
                                    lhsT=attTs[hh][:, st * P:(st + 1) * P],
                                    rhs=woTs[:, hh, mc * SC:(mc + 1) * SC],
                                    start=(hh == 0),
                                    stop=(hh == HPC - 1),
                                )
                        for mc in range(4):
                            # copies alternate DVE/ACT so neither serializes
                            if mc % 2 == 0:
                                nc.vector.tensor_copy(
                                    og[:, mc * SC:(mc + 1) * SC], ps_o[mc]
                                )
                            else:
                                nc.scalar.copy(
                                    og[:, mc * SC:(mc + 1) * SC], ps_o[mc]
                                )
                        nc.sync.dma_start(out[st * P:(st + 1) * P, :], og)

    nc.compile()
    _NC_CACHE[key] = nc
    return nc


def make_in_maps(x, wq, wk, wv, wo):
    x = np.asarray(x, dtype=np.float16)
    wq = np.asarray(wq, dtype=np.float16)
    wk = np.asarray(wk, dtype=np.float16)
    wv = np.asarray(wv, dtype=np.float16)
    wo = np.asarray(wo, dtype=np.float16)
    # mask[j, c] = 1 iff key j visible to query c within a diagonal block
    causal = np.triu(np.ones((P, P), dtype=np.float16))
    ones = np.ones((P, P), dtype=np.float16)
    in_maps = []
    for c in range(N_CORES):
        b, g = divmod(c, HPC)
        j0 = g * JC
        in_maps.append(
            {
                "xT": np.ascontiguousarray(x[b].T),
                "wqT": np.ascontiguousarray(wq[j0:j0 + JC].T),
                "wkT": np.ascontiguousarray(wk[j0:j0 + JC].T),
                "wvT": np.ascontiguousarray(wv[j0:j0 + JC].T),
                "woT": np.ascontiguousarray(wo[:, j0:j0 + JC].T),
                "mask": causal,
                "ones": ones,
            }
        )
    return in_maps


def combine_outputs(results):
    out = np.zeros((B, S, D), dtype=np.float32)
    for c in range(N_CORES):
        out[c // HPC] += np.asarray(results[c]["out"], dtype=np.float32)
    return out


def kernel(x, wq, wk, wv, wo):
    from concourse.bass_utils import run_bass_kernel_spmd

    nc = build_module()
    in_maps = make_in_maps(x, wq, wk, wv, wo)
    res = run_bass_kernel_spmd(nc, in_maps, list(range(N_CORES)))
    return combine_outputs(res.results)
